# revision 2
# baseline (speedup 1.0000x reference)
"""Trainium2 Bass kernel for nn_EnsembleClustering_62646392979777 — v2.

v1 was PE-sequencer-bound (~580 matmul instrs / iteration). v2:
  * Projection emitted TRANSPOSED: YT_b[n, oc] = XP_b^T @ W in 12 bf16
    matmuls of N=480 per batch (replaces v1's 72 [48,53] matmuls + the
    whole v-group stage).  v-groups are consumed straight from YT; k/p
    groups are PE-transposed per (group, head) slot with the proj bias
    fused into the PSUM->SBUF drain.
  * Both batches fused per rep body: stage-C free-stacks 16 (b,h)
    slices, halving per-iteration instruction count.
  * proj2 emitted transposed (z2T = Zf^T @ P2W, bf16 N=384): no
    transposes between proj2 and the upsample matmul.
  * Upsample as bf16 [49x3136] matmul (MUP is exact in bf16).
  * p-norms via free-dim reduce on bias-added YT p-cols (0 PE instrs);
    agent-pool mean folded into proj weights; v/vc bias folded into a
    single precomputed 2*bv tile (softmax rows sum to 1).
  * Pool engine (no PSUM access) takes the SBUF->SBUF side work.
"""
import sys
import numpy as np

sys.path.insert(0, "/opt/trn_rl_repo")

import ml_dtypes  # noqa: E402

import concourse.bass as bass  # noqa: E402
import concourse.tile as tile  # noqa: E402
from concourse import bacc, mybir  # noqa: E402
from concourse.bass_utils import run_bass_kernel_spmd  # noqa: E402
from concourse.masks import make_identity  # noqa: E402

F32 = mybir.dt.float32
BF16 = mybir.dt.bfloat16
AX = mybir.AxisListType
AF = mybir.ActivationFunctionType
OP = mybir.AluOpType

EPS = 1e-6
INV_SQRT_C = float(1.0 / np.sqrt(np.float32(48.0)))

_CACHE = {}


def _upsample_matrix():
    U = np.zeros((56, 7), dtype=np.float64)
    for o in range(56):
        src = (o + 0.5) / 8.0 - 0.5
        i0 = int(np.floor(src))
        t = src - i0
        U[o, min(max(i0, 0), 6)] += 1.0 - t
        U[o, min(max(i0 + 1, 0), 6)] += t
    U = U.astype(np.float32)
    return np.einsum("Oi,Pj->ijOP", U, U).reshape(49, 3136).copy()


def build_nc(reps=1, stage="full"):
    nc = bacc.Bacc("TRN2", target_bir_lowering=False, debug=False,
                   enable_asserts=False)

    x_d = nc.dram_tensor("x", [2, 384, 3136], F32, kind="ExternalInput").ap()
    pwt_d = nc.dram_tensor("pwt", [128, 3, 1920], BF16, kind="ExternalInput").ap()
    pbp_d = nc.dram_tensor("pbp", [49, 384], F32, kind="ExternalInput").ap()
    pbns_d = nc.dram_tensor("pbns", [48, 48], F32, kind="ExternalInput").ap()
    pbv_d = nc.dram_tensor("pbv", [68, 2, 384], F32, kind="ExternalInput").ap()
    al_d = nc.dram_tensor("alph", [1, 128], F32, kind="ExternalInput").ap()
    be_d = nc.dram_tensor("beta", [1, 128], F32, kind="ExternalInput").ap()
    p2w_d = nc.dram_tensor("p2w", [48, 8, 384], BF16, kind="ExternalInput").ap()
    p2b_d = nc.dram_tensor("p2b", [49, 384], F32, kind="ExternalInput").ap()
    mup_d = nc.dram_tensor("mup", [49, 3136], BF16, kind="ExternalInput").ap()
    y_d = nc.dram_tensor("y", [2, 384, 3136], BF16, kind="ExternalOutput").ap()

    with tile.TileContext(nc) as tc:
        with tc.tile_pool(name="w", bufs=1) as wp, \
             tc.tile_pool(name="xin", bufs=2) as xin, \
             tc.tile_pool(name="st", bufs=2) as st, \
             tc.tile_pool(name="outp", bufs=2) as outp, \
             tc.tile_pool(name="ps", bufs=2, space="PSUM") as ps:

            # ---------------- constants & weights ----------------
            ident = wp.tile([128, 128], F32, tag="ident")
            make_identity(nc, ident[:])
            ones_c = wp.tile([49, 1], F32, tag="ones_c")
            nc.vector.memset(ones_c[:], 1.0)
            ones_r = wp.tile([1, 128], F32, tag="ones_r")
            nc.vector.memset(ones_r[:], 1.0)

            PWT = wp.tile([128, 3, 1920], BF16, tag="pwt")
            nc.sync.dma_start(PWT[:], pwt_d)
            PBP = wp.tile([49, 384], F32, tag="pbp")
            nc.sync.dma_start(PBP[:], pbp_d)
            PBNS = wp.tile([48, 48], F32, tag="pbns")
            nc.sync.dma_start(PBNS[:], pbns_d)
            PBV = wp.tile([68, 2, 384], F32, tag="pbv")
            nc.sync.dma_start(PBV[:], pbv_d)
            P2W = wp.tile([48, 8, 384], BF16, tag="p2w")
            nc.sync.dma_start(P2W[:], p2w_d)
            P2B = wp.tile([49, 384], F32, tag="p2b")
            nc.sync.dma_start(P2B[:], p2b_d)
            MUP = wp.tile([49, 3136], BF16, tag="mup")
            nc.sync.dma_start(MUP[:], mup_d)
            AL1 = wp.tile([1, 128], F32, tag="al1")
            nc.sync.dma_start(AL1[:], al_d)
            BE1 = wp.tile([1, 128], F32, tag="be1")
            nc.sync.dma_start(BE1[:], be_d)

            # broadcast alpha/beta rows down 49 partitions (one-time)
            ALB = wp.tile([49, 128], F32, tag="alb")
            BEB = wp.tile([49, 128], F32, tag="beb")
            for src, dst in ((AL1, ALB), (BE1, BEB)):
                pt = ps.tile([49, 128], F32, tag="M")
                nc.tensor.matmul(pt[:], ones_r[:, :49], src[:], start=True, stop=True)
                nc.vector.tensor_copy(dst[:], pt[:])

            # alternating XP buffers; pad cols zeroed once
            XPb, XPBb = [], []
            for k in range(2):
                t = wp.tile([128, 3, 136], F32, tag=f"xp{k}")
                nc.vector.memset(t[:], 0.0)
                XPb.append(t)
                tb = wp.tile([128, 3, 136], BF16, tag=f"xpb{k}")
                nc.vector.memset(tb[:], 0.0)
                XPBb.append(tb)

            # ---------------- per-iteration pipeline ----------------
            for rep in range(reps):
                XP = XPb[rep % 2]
                XPB = XPBb[rep % 2]
                # ---- Stage A: load & pool both batches ----
                for b in range(2):
                    c0 = 68 * b
                    for j in range(3):
                        X = xin.tile([128, 3136], F32, tag="x")
                        nc.sync.dma_start(X[:], x_d[b, 128 * j:128 * (j + 1), :])
                        R2 = st.tile([128, 196], F32, tag="r2")
                        nc.vector.reduce_sum(
                            R2[:],
                            X[:].rearrange("p (oh hi ow wi) -> p oh ow hi wi",
                                           oh=14, hi=4, ow=14, wi=4),
                            axis=AX.XY)
                        nc.vector.reduce_sum(
                            XP[:, j, c0:c0 + 49],
                            R2[:].rearrange("p (oh hi ow wi) -> p oh ow hi wi",
                                            oh=7, hi=2, ow=7, wi=2),
                            axis=AX.XY)
                        nc.vector.reduce_sum(
                            XP[:, j, c0 + 64:c0 + 68],
                            R2[:].rearrange("p (oh hi ow wi) -> p oh ow hi wi",
                                            oh=2, hi=7, ow=2, wi=7),
                            axis=AX.XY)
                # cluster cols: raw 784-sums through W/64 -> correct by 64/784
                for b in range(2):
                    nc.vector.tensor_scalar_mul(
                        XP[:, :, 68 * b + 64:68 * b + 68],
                        XP[:, :, 68 * b + 64:68 * b + 68], 64.0 / 784.0)
                nc.gpsimd.tensor_copy(XPB[:], XP[:])

                if stage == "pool":
                    ri = rep % 7
                    nc.sync.dma_start(
                        y_d[0, 0:128, 408 * ri:408 * ri + 408],
                        XPB[:].rearrange("p a b -> p (a b)"))
                    continue

                # ---- Stage B: transposed projection YT_b [68, 1920] (raw) ----
                YT = []
                for b in range(2):
                    yt = st.tile([68, 1920], F32, tag=f"yt{b}")
                    for q in range(4):
                        pt = ps.tile([68, 480], F32, tag="Y")
                        for j in range(3):
                            nc.tensor.matmul(
                                pt[:],
                                XPB[:, j, 68 * b:68 * b + 68],
                                PWT[:, j, 480 * q:480 * (q + 1)],
                                start=(j == 0), stop=(j == 2))
                        if q % 2 == 0:
                            nc.vector.tensor_copy(yt[:, 480 * q:480 * (q + 1)], pt[:])
                        else:
                            nc.scalar.copy(yt[:, 480 * q:480 * (q + 1)], pt[:])
                    YT.append(yt)

                # biased p-cols (for p-norms) and vc (+2*bv) tiles, off-PSUM
                YTP = st.tile([49, 2, 384], F32, tag="ytp")
                VCB = st.tile([68, 2, 2, 384], F32, tag="vcb")
                for b in range(2):
                    nc.gpsimd.tensor_tensor(YTP[:, b, :], YT[b][0:49, 0:384],
                                            PBP[:], OP.add)
                    for i in range(2):
                        voc = (2 + 2 * i) * 384
                        nc.gpsimd.tensor_tensor(VCB[64:68, b, i, :],
                                                YT[b][64:68, voc:voc + 384],
                                                PBV[64:68, i, :], OP.add)

                # ---- slots: k0/k1/p transposed to c-on-partitions + bias ----
                KSb = st.tile([48, 48, 68], F32, tag="ksb")
                si = 0
                for b in range(2):
                    for gi, g in enumerate((1, 3, 0)):  # oc groups k0, k1, p
                        for h in range(8):
                            t = b * 24 + gi * 8 + h
                            w68 = 68 if gi < 2 else 49
                            pt = ps.tile([48, 68], F32, tag="S")
                            if gi < 2:
                                nc.tensor.transpose(
                                    pt[:],
                                    YT[b][:, g * 384 + 48 * h:g * 384 + 48 * (h + 1)],
                                    ident[:68, :68])
                                bcol = b * 16 + (0 if g == 1 else 8) + h
                            else:
                                nc.tensor.transpose(
                                    pt[:, 0:49],
                                    YTP[:, b, 48 * h:48 * (h + 1)],
                                    ident[:49, :49])
                                bcol = 32 + b * 8 + h
                            if si % 2 == 0:
                                nc.scalar.activation(
                                    KSb[:, t, 0:w68], pt[:, 0:w68], AF.Identity,
                                    bias=PBNS[:, bcol:bcol + 1], scale=1.0)
                            else:
                                nc.vector.tensor_tensor(
                                    KSb[:, t, 0:w68], pt[:, 0:w68],
                                    PBNS[:, bcol:bcol + 1].to_broadcast((48, w68)),
                                    OP.add)
                            si += 1

                def kslot(b, gi, h):
                    return KSb[:, b * 24 + gi * 8 + h, :]

                # ---- p-norms from biased YTP (free-dim reduce) ----
                SQP = st.tile([49, 2, 384], F32, tag="sqp")
                nc.gpsimd.tensor_tensor(
                    SQP[:].rearrange("p a b -> p (a b)"),
                    YTP[:].rearrange("p a b -> p (a b)"),
                    YTP[:].rearrange("p a b -> p (a b)"), OP.mult)
                RP = st.tile([49, 16], F32, tag="rp")
                nc.vector.reduce_sum(
                    RP[:], SQP[:].rearrange("p a (g c) -> p (a g) c", c=48),
                    axis=AX.X)
                nc.scalar.activation(RP[:], RP[:], AF.Sqrt)
                nc.vector.tensor_scalar_add(RP[:], RP[:], EPS)
                nc.vector.reciprocal(RP[:], RP[:])

                # ---- scores ----
                S0f = st.tile([4, 16, 49], F32, tag="s0")
                S1f = st.tile([49, 16, 4], F32, tag="s1")
                for b in range(2):
                    for h in range(8):
                        s = b * 8 + h
                        k0 = kslot(b, 0, h)
                        k1 = kslot(b, 1, h)
                        pt = ps.tile([4, 49], F32, tag="S")
                        nc.tensor.matmul(pt[:], k0[:, 64:68], k0[:, 0:49],
                                         start=True, stop=True)
                        nc.vector.tensor_scalar_mul(S0f[:, s, :], pt[:], INV_SQRT_C)
                        pt2 = ps.tile([49, 4], F32, tag="S")
                        nc.tensor.matmul(pt2[:], k1[:, 0:49], k1[:, 64:68],
                                         start=True, stop=True)
                        nc.scalar.mul(S1f[:, s, :], pt2[:], INV_SQRT_C)

                # softmax0 over tokens (module 0), in place
                M0 = st.tile([4, 16], F32, tag="m0")
                nc.vector.reduce_max(M0[:], S0f[:], axis=AX.X, negate=True)
                nc.vector.tensor_tensor(S0f[:], S0f[:],
                                        M0[:, :, None].to_broadcast((4, 16, 49)),
                                        OP.add)
                nc.scalar.activation(S0f[:], S0f[:], AF.Exp)
                SM0 = st.tile([4, 16], F32, tag="sm0")
                nc.vector.reduce_sum(SM0[:], S0f[:], axis=AX.X)
                nc.vector.reciprocal(SM0[:], SM0[:])
                A0 = S0f
                nc.gpsimd.tensor_tensor(A0[:], S0f[:],
                                        SM0[:, :, None].to_broadcast((4, 16, 49)),
                                        OP.mult)

                # softmax1 over clusters (module 1), in place
                M1 = st.tile([49, 16], F32, tag="m1")
                nc.vector.reduce_max(M1[:], S1f[:], axis=AX.X, negate=True)
                nc.gpsimd.tensor_tensor(S1f[:], S1f[:],
                                        M1[:, :, None].to_broadcast((49, 16, 4)),
                                        OP.add)
                nc.scalar.activation(S1f[:], S1f[:], AF.Exp)
                SM1 = st.tile([49, 16], F32, tag="sm1")
                nc.vector.reduce_sum(SM1[:], S1f[:], axis=AX.X)
                nc.vector.reciprocal(SM1[:], SM1[:])
                A1T = S1f
                nc.gpsimd.tensor_tensor(A1T[:], S1f[:],
                                        SM1[:, :, None].to_broadcast((49, 16, 4)),
                                        OP.mult)

                # A0 -> token-on-partition layout
                A0T = st.tile([49, 16, 4], F32, tag="a0t")
                for s in range(16):
                    pt = ps.tile([49, 4], F32, tag="S")
                    nc.tensor.transpose(pt[:], A0[:, s, :], ident[:4, :4])
                    if si % 2 == 0:
                        nc.scalar.copy(A0T[:, s, :], pt[:])
                    else:
                        nc.vector.tensor_copy(A0T[:, s, :], pt[:])
                    si += 1

                # fuzzy normalizer 1/(sum_n memb + eps), PE-broadcast
                ptd = ps.tile([1, 64], F32, tag="S")
                nc.tensor.matmul(ptd[:], ones_c[:],
                                 A1T[:].rearrange("p a b -> p (a b)"),
                                 start=True, stop=True)
                DE = st.tile([1, 64], F32, tag="de")
                nc.vector.tensor_scalar_add(DE[:], ptd[:], EPS)
                nc.vector.reciprocal(DE[:], DE[:])
                ptb = ps.tile([49, 64], F32, tag="M")
                nc.tensor.matmul(ptb[:], ones_r[:, :49], DE[:], start=True, stop=True)
                A1N = A1T
                nc.vector.tensor_tensor(A1N[:].rearrange("p a b -> p (a b)"),
                                        A1T[:].rearrange("p a b -> p (a b)"),
                                        ptb[:], OP.mult)

                # ---- agg (+vc+2bv): AGGf [36, 2i, 384], b at base 32b ----
                AGGf = st.tile([36, 2, 384], F32, tag="aggf")
                for b in range(2):
                    for i in range(2):
                        voc = (2 + 2 * i) * 384
                        pt = ps.tile([4, 384], F32, tag="M")
                        for h in range(8):
                            AT = A0T if i == 0 else A1N
                            nc.tensor.matmul(
                                pt[:, 48 * h:48 * (h + 1)],
                                AT[:, b * 8 + h, :],
                                YT[b][0:49, voc + 48 * h:voc + 48 * (h + 1)],
                                start=True, stop=True)
                        nc.vector.tensor_tensor(AGGf[32 * b:32 * b + 4, i, :], pt[:],
                                                VCB[64:68, b, i, :], OP.add)

                # agg norms -> AGGN (garbage partitions 4-31 harmless)
                SQ = st.tile([36, 768], F32, tag="sq")
                nc.gpsimd.tensor_tensor(SQ[:], AGGf[:].rearrange("p a c -> p (a c)"),
                                        AGGf[:].rearrange("p a c -> p (a c)"),
                                        OP.mult)
                SS = st.tile([36, 16], F32, tag="ss")
                nc.vector.reduce_sum(SS[:], SQ[:].rearrange("p (g c) -> p g c", c=48),
                                     axis=AX.X)
                nc.scalar.activation(SS[:], SS[:], AF.Sqrt)
                nc.vector.tensor_scalar_add(SS[:], SS[:], EPS)
                nc.vector.reciprocal(SS[:], SS[:])
                AGGN = SQ
                nc.gpsimd.tensor_tensor(
                    AGGN[:].rearrange("p (g c) -> p g c", c=48),
                    AGGf[:].rearrange("p a (g c) -> p (a g) c", c=48),
                    SS[:, :, None].to_broadcast((36, 16, 48)), OP.mult)

                # AGGN slices -> c-on-partitions AGGNT [48, (b,h), 8]
                AGGNT = st.tile([48, 16, 8], F32, tag="aggnt")
                for b in range(2):
                    for i in range(2):
                        for h in range(8):
                            pt = ps.tile([48, 4], F32, tag="S")
                            nc.tensor.transpose(
                                pt[:],
                                AGGN[32 * b:32 * b + 4, 384 * i + 48 * h:384 * i + 48 * (h + 1)],
                                ident[32 * b:32 * b + 4, 32 * b:32 * b + 4])
                            dst = AGGNT[:, b * 8 + h, 4 * i:4 * (i + 1)]
                            if si % 2 == 0:
                                nc.scalar.copy(dst, pt[:])
                            else:
                                nc.vector.tensor_copy(dst, pt[:])
                            si += 1

                # sim^T [49, (b,h,m)] = p^T @ aggnT ; rp / alpha / beta
                ptm = ps.tile([49, 128], F32, tag="M")
                for b in range(2):
                    for h in range(8):
                        s = b * 8 + h
                        nc.tensor.matmul(ptm[:, 8 * s:8 * (s + 1)],
                                         kslot(b, 2, h)[:, 0:49],
                                         AGGNT[:, s, :], start=True, stop=True)
                SIMT = st.tile([49, 16, 8], F32, tag="simt")
                nc.vector.tensor_tensor(SIMT[:],
                                        ptm[:].rearrange("p (a b) -> p a b", b=8),
                                        RP[:, :, None].to_broadcast((49, 16, 8)),
                                        OP.mult)
                nc.gpsimd.tensor_tensor(SIMT[:].rearrange("p a b -> p (a b)"),
                                        SIMT[:].rearrange("p a b -> p (a b)"),
                                        ALB[:], OP.mult)
                nc.gpsimd.tensor_tensor(SIMT[:].rearrange("p a b -> p (a b)"),
                                        SIMT[:].rearrange("p a b -> p (a b)"),
                                        BEB[:], OP.add)

                # assignment softmax over the 8 clusters, in place
                MM = st.tile([49, 16], F32, tag="mm")
                nc.vector.reduce_max(MM[:], SIMT[:], axis=AX.X, negate=True)
                nc.vector.tensor_tensor(SIMT[:], SIMT[:],
                                        MM[:, :, None].to_broadcast((49, 16, 8)),
                                        OP.add)
                nc.scalar.activation(SIMT[:], SIMT[:], AF.Exp)
                SMS = st.tile([49, 16], F32, tag="sms")
                nc.vector.reduce_sum(SMS[:], SIMT[:], axis=AX.X)
                nc.vector.reciprocal(SMS[:], SMS[:])
                ASGT = SIMT
                nc.gpsimd.tensor_tensor(ASGT[:], SIMT[:],
                                        SMS[:, :, None].to_broadcast((49, 16, 8)),
                                        OP.mult)

                # assignment -> m-on-partitions (b at base 32b)
                ASG = st.tile([36, 2, 8, 49], F32, tag="asg")
                for s in range(16):
                    b, h = divmod(s, 8)
                    for i in range(2):
                        pt = ps.tile([4, 49], F32, tag="S")
                        nc.tensor.transpose(pt[:], ASGT[:, s, 4 * i:4 * (i + 1)],
                                            ident[:49, :49])
                        dst = ASG[32 * b:32 * b + 4, i, h, :]
                        if si % 2 == 0:
                            nc.scalar.copy(dst, pt[:])
                        else:
                            nc.vector.tensor_copy(dst, pt[:])
                        si += 1

                # out_low z [48, h, (b*49)] = agg^T @ assignment  (bf16 out)
                Zf = st.tile([48, 8, 98], BF16, tag="zf")
                for b in range(2):
                    for h in range(8):
                        pt = ps.tile([48, 49], F32, tag="S")
                        nc.tensor.matmul(
                            pt[:],
                            AGGf[32 * b:32 * b + 4, 0, 48 * h:48 * (h + 1)],
                            ASG[32 * b:32 * b + 4, 0, h, :],
                            start=True, stop=False)
                        nc.tensor.matmul(
                            pt[:],
                            AGGf[32 * b:32 * b + 4, 1, 48 * h:48 * (h + 1)],
                            ASG[32 * b:32 * b + 4, 1, h, :],
                            start=False, stop=True)
                        dst = Zf[:, h, 49 * b:49 * (b + 1)]
                        if si % 2 == 0:
                            nc.scalar.copy(dst, pt[:])
                        else:
                            nc.vector.tensor_copy(dst, pt[:])
                        si += 1

                # proj2 transposed: z2T_b [49, 384] = Zf_b^T @ P2W (+bias)
                Z2T = []
                for b in range(2):
                    pt = ps.tile([49, 384], F32, tag="M")
                    for h in range(8):
                        nc.tensor.matmul(pt[:],
                                         Zf[:, h, 49 * b:49 * (b + 1)],
                                         P2W[:, h, :],
                                         start=(h == 0), stop=(h == 7))
                    z2t = st.tile([49, 384], BF16, tag=f"z2t{b}")
                    nc.vector.tensor_tensor(z2t[:], pt[:], P2B[:], OP.add)
                    Z2T.append(z2t)

                if stage == "noup":
                    ri = rep % 8
                    nc.sync.dma_start(y_d[0, 0:49, 384 * ri:384 * (ri + 1)],
                                      Z2T[0][:])
                    continue

                # upsample: [128, 3136] = z2T^T @ MUP (bf16), stage via SBUF
                for b in range(2):
                    for oi in range(3):
                        OUT = outp.tile([128, 3136], BF16, tag="out")
                        for nt in range(7):
                            pt = ps.tile([128, 448], F32, tag="U")
                            nc.tensor.matmul(
                                pt[:],
                                Z2T[b][:, 128 * oi:128 * (oi + 1)],
                                MUP[:, 448 * nt:448 * (nt + 1)],
                                start=True, stop=True)
                            dst = OUT[:, 448 * nt:448 * (nt + 1)]
                            if si % 5 < 3:
                                nc.scalar.copy(dst, pt[:])
                            else:
                                nc.vector.tensor_copy(dst, pt[:])
                            si += 1
                        nc.sync.dma_start(y_d[b, 128 * oi:128 * (oi + 1), :],
                                          OUT[:])

    nc.compile()
    return nc


def _prep_weights(proj_w, proj_b, sim_alpha, sim_beta, proj2_w, proj2_b):
    pwT = np.ascontiguousarray(proj_w.T) / 64.0                # [384, 1920]
    pwt = np.ascontiguousarray(pwT.reshape(3, 128, 1920).transpose(1, 0, 2))
    pbp = np.broadcast_to(proj_b[None, 0:384], (49, 384)).copy()
    # slot bias cols: 0-15 k0 (b0 then b1), 16-31 k1, 32-47 p
    pbns = np.zeros((48, 48), np.float32)
    for b in range(2):
        for h in range(8):
            pbns[:, b * 16 + h] = proj_b[384 + 48 * h:384 + 48 * (h + 1)]
            pbns[:, b * 16 + 8 + h] = proj_b[1152 + 48 * h:1152 + 48 * (h + 1)]
            pbns[:, 32 + b * 8 + h] = proj_b[48 * h:48 * (h + 1)]
    pbv = np.zeros((68, 2, 384), np.float32)
    pbv[64:68, 0, :] = 2.0 * proj_b[768:1152][None, :]
    pbv[64:68, 1, :] = 2.0 * proj_b[1536:1920][None, :]
    alph = np.tile(sim_alpha, 16)[None, :].astype(np.float32)
    beta = np.tile(sim_beta, 16)[None, :].astype(np.float32)
    p2wT = np.ascontiguousarray(proj2_w.T)                     # [384, 384]
    p2w = np.ascontiguousarray(p2wT.reshape(8, 48, 384).transpose(1, 0, 2))
    p2b = np.broadcast_to(proj2_b[None, :], (49, 384)).copy()
    mup = _upsample_matrix()
    return {"pwt": pwt.astype(ml_dtypes.bfloat16), "pbp": pbp, "pbns": pbns,
            "pbv": np.ascontiguousarray(pbv), "alph": alph, "beta": beta,
            "p2w": p2w.astype(ml_dtypes.bfloat16), "p2b": p2b,
            "mup": mup.astype(ml_dtypes.bfloat16)}


def kernel(x, proj_w, proj_b, sim_alpha, sim_beta, proj2_w, proj2_b):
    x = np.asarray(x, np.float32)
    proj_w = np.asarray(proj_w, np.float32)
    proj_b = np.asarray(proj_b, np.float32)
    sim_alpha = np.asarray(sim_alpha, np.float32)
    sim_beta = np.asarray(sim_beta, np.float32)
    proj2_w = np.asarray(proj2_w, np.float32)
    proj2_b = np.asarray(proj2_b, np.float32)

    if "nc" not in _CACHE:
        _CACHE["nc"] = build_nc()
    nc = _CACHE["nc"]

    w = _prep_weights(proj_w, proj_b, sim_alpha, sim_beta, proj2_w, proj2_b)
    B = x.shape[0]
    xr = x.reshape(8, B // 8, 384, 3136)
    in_maps = [dict(w, x=np.ascontiguousarray(xr[c])) for c in range(8)]

    res = run_bass_kernel_spmd(nc, in_maps, core_ids=list(range(8)))
    out = np.concatenate([np.asarray(r["y"]) for r in res.results], axis=0)
    return out.reshape(16, 384, 56, 56).astype(np.float32)


if __name__ == "__main__":
    rng = np.random.default_rng(0)
    inputs = {
        "x": rng.standard_normal((16, 384, 56, 56), dtype=np.float32),
        "proj_w": rng.standard_normal((1920, 384), dtype=np.float32) * 384 ** -0.5,
        "proj_b": rng.standard_normal(1920).astype(np.float32) * 0.1,
        "sim_alpha": np.ones(8, np.float32),
        "sim_beta": np.zeros(8, np.float32),
        "proj2_w": rng.standard_normal((384, 384), dtype=np.float32) * 384 ** -0.5,
        "proj2_b": rng.standard_normal(384).astype(np.float32) * 0.1,
    }
    out = kernel(**inputs)
    print("kernel ran, output", out.shape, out.dtype, float(np.abs(out).max()))


# revision 5
# speedup vs baseline: 1.5279x; 1.5279x over previous
"""Trainium2 Bass kernel for nn_EnsembleClustering_62646392979777 — v2.

v1 was PE-sequencer-bound (~580 matmul instrs / iteration). v2:
  * Projection emitted TRANSPOSED: YT_b[n, oc] = XP_b^T @ W in 12 bf16
    matmuls of N=480 per batch (replaces v1's 72 [48,53] matmuls + the
    whole v-group stage).  v-groups are consumed straight from YT; k/p
    groups are PE-transposed per (group, head) slot with the proj bias
    fused into the PSUM->SBUF drain.
  * Both batches fused per rep body: stage-C free-stacks 16 (b,h)
    slices, halving per-iteration instruction count.
  * proj2 emitted transposed (z2T = Zf^T @ P2W, bf16 N=384): no
    transposes between proj2 and the upsample matmul.
  * Upsample as bf16 [49x3136] matmul (MUP is exact in bf16).
  * p-norms via free-dim reduce on bias-added YT p-cols (0 PE instrs);
    agent-pool mean folded into proj weights; v/vc bias folded into a
    single precomputed 2*bv tile (softmax rows sum to 1).
  * Pool engine (no PSUM access) takes the SBUF->SBUF side work.
"""
import sys
import numpy as np

sys.path.insert(0, "/opt/trn_rl_repo")

import ml_dtypes  # noqa: E402

import concourse.bass as bass  # noqa: E402
import concourse.tile as tile  # noqa: E402
from concourse import bacc, mybir  # noqa: E402
from concourse.bass_utils import run_bass_kernel_spmd  # noqa: E402
from concourse.masks import make_identity  # noqa: E402

F32 = mybir.dt.float32
BF16 = mybir.dt.bfloat16
AX = mybir.AxisListType
AF = mybir.ActivationFunctionType
OP = mybir.AluOpType

EPS = 1e-6
INV_SQRT_C = float(1.0 / np.sqrt(np.float32(48.0)))

_CACHE = {}


def _upsample_matrix():
    U = np.zeros((56, 7), dtype=np.float64)
    for o in range(56):
        src = (o + 0.5) / 8.0 - 0.5
        i0 = int(np.floor(src))
        t = src - i0
        U[o, min(max(i0, 0), 6)] += 1.0 - t
        U[o, min(max(i0 + 1, 0), 6)] += t
    U = U.astype(np.float32)
    return np.einsum("Oi,Pj->ijOP", U, U).reshape(49, 3136).copy()


def build_nc(reps=1, stage="full"):
    nc = bacc.Bacc("TRN2", target_bir_lowering=False, debug=False,
                   enable_asserts=False)

    x_d = nc.dram_tensor("x", [2, 384, 3136], F32, kind="ExternalInput").ap()
    pwt_d = nc.dram_tensor("pwt", [128, 3, 1920], BF16, kind="ExternalInput").ap()
    pbp_d = nc.dram_tensor("pbp", [49, 384], F32, kind="ExternalInput").ap()
    pbns_d = nc.dram_tensor("pbns", [48, 48], F32, kind="ExternalInput").ap()
    pbv_d = nc.dram_tensor("pbv", [68, 2, 384], F32, kind="ExternalInput").ap()
    al_d = nc.dram_tensor("alph", [1, 128], F32, kind="ExternalInput").ap()
    be_d = nc.dram_tensor("beta", [1, 128], F32, kind="ExternalInput").ap()
    p2w_d = nc.dram_tensor("p2w", [48, 8, 384], BF16, kind="ExternalInput").ap()
    p2b_d = nc.dram_tensor("p2b", [49, 384], F32, kind="ExternalInput").ap()
    mup_d = nc.dram_tensor("mup", [49, 3136], BF16, kind="ExternalInput").ap()
    y_d = nc.dram_tensor("y", [2, 384, 3136], BF16, kind="ExternalOutput").ap()

    with tile.TileContext(nc) as tc:
        with tc.tile_pool(name="w", bufs=1) as wp, \
             tc.tile_pool(name="xin", bufs=3) as xin, \
             tc.tile_pool(name="st", bufs=2) as st, \
             tc.tile_pool(name="outp", bufs=2) as outp, \
             tc.tile_pool(name="ps", bufs=2, space="PSUM") as ps:

            # ---------------- constants & weights ----------------
            ident = wp.tile([128, 128], F32, tag="ident")
            make_identity(nc, ident[:])
            ones_c = wp.tile([49, 1], F32, tag="ones_c")
            nc.vector.memset(ones_c[:], 1.0)
            ones_r = wp.tile([1, 128], F32, tag="ones_r")
            nc.vector.memset(ones_r[:], 1.0)

            PWT = wp.tile([128, 3, 1920], BF16, tag="pwt")
            nc.sync.dma_start(PWT[:], pwt_d)
            PBP = wp.tile([49, 384], F32, tag="pbp")
            nc.sync.dma_start(PBP[:], pbp_d)
            PBNS = wp.tile([48, 48], F32, tag="pbns")
            nc.sync.dma_start(PBNS[:], pbns_d)
            PBV = wp.tile([68, 2, 384], F32, tag="pbv")
            nc.sync.dma_start(PBV[:], pbv_d)
            P2W = wp.tile([48, 8, 384], BF16, tag="p2w")
            nc.sync.dma_start(P2W[:], p2w_d)
            P2B = wp.tile([49, 384], F32, tag="p2b")
            nc.sync.dma_start(P2B[:], p2b_d)
            MUP = wp.tile([49, 3136], BF16, tag="mup")
            nc.sync.dma_start(MUP[:], mup_d)
            AL1 = wp.tile([1, 128], F32, tag="al1")
            nc.sync.dma_start(AL1[:], al_d)
            BE1 = wp.tile([1, 128], F32, tag="be1")
            nc.sync.dma_start(BE1[:], be_d)

            # broadcast alpha/beta rows down 49 partitions (one-time)
            ALB = wp.tile([49, 128], F32, tag="alb")
            BEB = wp.tile([49, 128], F32, tag="beb")
            for src, dst in ((AL1, ALB), (BE1, BEB)):
                pt = ps.tile([49, 128], F32, tag="M")
                nc.tensor.matmul(pt[:], ones_r[:, :49], src[:], start=True, stop=True)
                nc.vector.tensor_copy(dst[:], pt[:])

            # alternating XP buffers; pad cols zeroed once
            XPb, XPBb = [], []
            for k in range(2):
                t = wp.tile([128, 3, 136], F32, tag=f"xp{k}")
                nc.vector.memset(t[:], 0.0)
                XPb.append(t)
                tb = wp.tile([128, 3, 136], BF16, tag=f"xpb{k}")
                nc.vector.memset(tb[:], 0.0)
                XPBb.append(tb)

            # ---------------- per-iteration pipeline ----------------
            I32 = mybir.dt.int32

            def pool_rsqrt(dst, n):
                # dst[:n partitions, 16] f32 := 1/sqrt(dst + 1e-12), computed
                # on the Pool engine (magic initial guess + 2 Newton steps)
                # to keep Ln/Sqrt off the Activation engine (table thrash).
                X = st.tile([49, 16], F32, tag="rsq_x")
                Y = st.tile([49, 16], F32, tag="rsq_y")
                T = st.tile([49, 16], F32, tag="rsq_t")
                x, y, t, d = X[0:n], Y[0:n], T[0:n], dst
                nc.gpsimd.tensor_scalar_add(x, d, 1e-12)
                nc.vector.tensor_scalar(
                    y.bitcast(I32), x.bitcast(I32), 1, 0,
                    op0=OP.logical_shift_right, op1=OP.bypass)
                nc.vector.tensor_scalar(
                    y.bitcast(I32), y.bitcast(I32), -1, 0x5f3759df,
                    op0=OP.mult, op1=OP.add)
                for _ in range(2):
                    nc.gpsimd.tensor_tensor(t, y, y, OP.mult)
                    nc.gpsimd.tensor_tensor(t, t, x, OP.mult)
                    nc.gpsimd.tensor_scalar_mul(t, t, -0.5)
                    nc.gpsimd.tensor_scalar_add(t, t, 1.5)
                    nc.gpsimd.tensor_tensor(y, y, t, OP.mult)
                nc.gpsimd.tensor_copy(d, y)
            for rep in range(reps):
                XP = XPb[rep % 2]
                XPB = XPBb[rep % 2]
                # ---- Stage A: load & pool both batches ----
                for b in range(2):
                    c0 = 68 * b
                    for j in range(3):
                        X = xin.tile([128, 3136], BF16, tag="x")
                        nc.gpsimd.dma_start(X[:], x_d[b, 128 * j:128 * (j + 1), :])
                        R2 = st.tile([128, 196], F32, tag="r2")
                        nc.vector.reduce_sum(
                            R2[:],
                            X[:].rearrange("p (oh hi ow wi) -> p oh ow hi wi",
                                           oh=14, hi=4, ow=14, wi=4),
                            axis=AX.XY)
                        nc.vector.reduce_sum(
                            XP[:, j, c0:c0 + 49],
                            R2[:].rearrange("p (oh hi ow wi) -> p oh ow hi wi",
                                            oh=7, hi=2, ow=7, wi=2),
                            axis=AX.XY)
                        nc.vector.reduce_sum(
                            XP[:, j, c0 + 64:c0 + 68],
                            R2[:].rearrange("p (oh hi ow wi) -> p oh ow hi wi",
                                            oh=2, hi=7, ow=2, wi=7),
                            axis=AX.XY)
                # cluster cols: raw 784-sums through W/64 -> correct by 64/784
                for b in range(2):
                    nc.vector.tensor_scalar_mul(
                        XP[:, :, 68 * b + 64:68 * b + 68],
                        XP[:, :, 68 * b + 64:68 * b + 68], 64.0 / 784.0)
                nc.gpsimd.tensor_copy(XPB[:], XP[:])

                if stage == "pool":
                    ri = rep % 7
                    nc.sync.dma_start(
                        y_d[0, 0:128, 408 * ri:408 * ri + 408],
                        XPB[:].rearrange("p a b -> p (a b)"))
                    continue

                # ---- Stage B: transposed projection YT_b [68, 1920] (raw) ----
                YT = []
                for b in range(2):
                    yt = st.tile([68, 1920], F32, tag=f"yt{b}")
                    for q in range(4):
                        pt = ps.tile([68, 480], F32, tag="Y")
                        for j in range(3):
                            nc.tensor.matmul(
                                pt[:],
                                XPB[:, j, 68 * b:68 * b + 68],
                                PWT[:, j, 480 * q:480 * (q + 1)],
                                start=(j == 0), stop=(j == 2))
                        if q % 2 == 0:
                            nc.vector.tensor_copy(yt[:, 480 * q:480 * (q + 1)], pt[:])
                        else:
                            nc.scalar.copy(yt[:, 480 * q:480 * (q + 1)], pt[:])
                    YT.append(yt)

                # biased p-cols (for p-norms) and vc (+2*bv) tiles, off-PSUM
                YTP = st.tile([49, 2, 384], F32, tag="ytp")
                VCB = st.tile([68, 2, 2, 384], F32, tag="vcb")
                for b in range(2):
                    nc.gpsimd.tensor_tensor(YTP[:, b, :], YT[b][0:49, 0:384],
                                            PBP[:], OP.add)
                    for i in range(2):
                        voc = (2 + 2 * i) * 384
                        nc.gpsimd.tensor_tensor(VCB[64:68, b, i, :],
                                                YT[b][64:68, voc:voc + 384],
                                                PBV[64:68, i, :], OP.add)

                # ---- slots: k0/k1/p transposed to c-on-partitions + bias ----
                KSb = st.tile([48, 48, 68], F32, tag="ksb")
                si = 0
                for b in range(2):
                    for gi, g in enumerate((1, 3, 0)):  # oc groups k0, k1, p
                        for h in range(8):
                            t = b * 24 + gi * 8 + h
                            w68 = 68 if gi < 2 else 49
                            pt = ps.tile([48, 68], F32, tag="S")
                            if gi < 2:
                                nc.tensor.transpose(
                                    pt[:],
                                    YT[b][:, g * 384 + 48 * h:g * 384 + 48 * (h + 1)],
                                    ident[:68, :68])
                                bcol = b * 16 + (0 if g == 1 else 8) + h
                            else:
                                nc.tensor.transpose(
                                    pt[:, 0:49],
                                    YTP[:, b, 48 * h:48 * (h + 1)],
                                    ident[:49, :49])
                                bcol = 32 + b * 8 + h
                            if si % 2 == 0:
                                nc.scalar.activation(
                                    KSb[:, t, 0:w68], pt[:, 0:w68], AF.Identity,
                                    bias=PBNS[:, bcol:bcol + 1], scale=1.0)
                            else:
                                nc.vector.tensor_tensor(
                                    KSb[:, t, 0:w68], pt[:, 0:w68],
                                    PBNS[:, bcol:bcol + 1].to_broadcast((48, w68)),
                                    OP.add)
                            si += 1

                def kslot(b, gi, h):
                    return KSb[:, b * 24 + gi * 8 + h, :]

                # ---- p-norms from biased YTP (free-dim reduce) ----
                SQP = st.tile([49, 2, 384], F32, tag="sqp")
                nc.gpsimd.tensor_tensor(
                    SQP[:].rearrange("p a b -> p (a b)"),
                    YTP[:].rearrange("p a b -> p (a b)"),
                    YTP[:].rearrange("p a b -> p (a b)"), OP.mult)
                RP = st.tile([49, 16], F32, tag="rp")
                nc.vector.reduce_sum(
                    RP[:], SQP[:].rearrange("p a (g c) -> p (a g) c", c=48),
                    axis=AX.X)
                nc.scalar.activation(RP[:], RP[:], AF.Sqrt)
                nc.vector.tensor_scalar_add(RP[:], RP[:], EPS)
                nc.vector.reciprocal(RP[:], RP[:])

                # ---- scores ----
                S0f = st.tile([4, 16, 49], F32, tag="s0")
                S1f = st.tile([49, 16, 4], F32, tag="s1")
                for b in range(2):
                    for h in range(8):
                        s = b * 8 + h
                        k0 = kslot(b, 0, h)
                        k1 = kslot(b, 1, h)
                        pt = ps.tile([4, 49], F32, tag="S")
                        nc.tensor.matmul(pt[:], k0[:, 64:68], k0[:, 0:49],
                                         start=True, stop=True)
                        nc.vector.tensor_scalar_mul(S0f[:, s, :], pt[:], INV_SQRT_C)
                        pt2 = ps.tile([49, 4], F32, tag="S")
                        nc.tensor.matmul(pt2[:], k1[:, 0:49], k1[:, 64:68],
                                         start=True, stop=True)
                        nc.scalar.mul(S1f[:, s, :], pt2[:], INV_SQRT_C)

                # softmax0 over tokens (module 0), in place
                M0 = st.tile([4, 16], F32, tag="m0")
                nc.vector.reduce_max(M0[:], S0f[:], axis=AX.X, negate=True)
                nc.vector.tensor_tensor(S0f[:], S0f[:],
                                        M0[:, :, None].to_broadcast((4, 16, 49)),
                                        OP.add)
                nc.scalar.activation(S0f[:], S0f[:], AF.Exp)
                SM0 = st.tile([4, 16], F32, tag="sm0")
                nc.vector.reduce_sum(SM0[:], S0f[:], axis=AX.X)
                nc.vector.reciprocal(SM0[:], SM0[:])
                A0 = S0f
                nc.gpsimd.tensor_tensor(A0[:], S0f[:],
                                        SM0[:, :, None].to_broadcast((4, 16, 49)),
                                        OP.mult)

                # softmax1 over clusters (module 1), in place
                M1 = st.tile([49, 16], F32, tag="m1")
                nc.vector.reduce_max(M1[:], S1f[:], axis=AX.X, negate=True)
                nc.gpsimd.tensor_tensor(S1f[:], S1f[:],
                                        M1[:, :, None].to_broadcast((49, 16, 4)),
                                        OP.add)
                nc.scalar.activation(S1f[:], S1f[:], AF.Exp)
                SM1 = st.tile([49, 16], F32, tag="sm1")
                nc.vector.reduce_sum(SM1[:], S1f[:], axis=AX.X)
                nc.vector.reciprocal(SM1[:], SM1[:])
                A1T = S1f
                nc.gpsimd.tensor_tensor(A1T[:], S1f[:],
                                        SM1[:, :, None].to_broadcast((49, 16, 4)),
                                        OP.mult)

                # A0 -> token-on-partition layout
                A0T = st.tile([49, 16, 4], F32, tag="a0t")
                for s in range(16):
                    pt = ps.tile([49, 4], F32, tag="S")
                    nc.tensor.transpose(pt[:], A0[:, s, :], ident[:4, :4])
                    if si % 2 == 0:
                        nc.scalar.copy(A0T[:, s, :], pt[:])
                    else:
                        nc.vector.tensor_copy(A0T[:, s, :], pt[:])
                    si += 1

                # fuzzy normalizer 1/(sum_n memb + eps), PE-broadcast
                ptd = ps.tile([1, 64], F32, tag="S")
                nc.tensor.matmul(ptd[:], ones_c[:],
                                 A1T[:].rearrange("p a b -> p (a b)"),
                                 start=True, stop=True)
                DE = st.tile([1, 64], F32, tag="de")
                nc.vector.tensor_scalar_add(DE[:], ptd[:], EPS)
                nc.vector.reciprocal(DE[:], DE[:])
                ptb = ps.tile([49, 64], F32, tag="M")
                nc.tensor.matmul(ptb[:], ones_r[:, :49], DE[:], start=True, stop=True)
                A1N = A1T
                nc.vector.tensor_tensor(A1N[:].rearrange("p a b -> p (a b)"),
                                        A1T[:].rearrange("p a b -> p (a b)"),
                                        ptb[:], OP.mult)

                # ---- agg (+vc+2bv): AGGf [36, 2i, 384], b at base 32b ----
                AGGf = st.tile([36, 2, 384], F32, tag="aggf")
                for b in range(2):
                    for i in range(2):
                        voc = (2 + 2 * i) * 384
                        pt = ps.tile([4, 384], F32, tag="M")
                        for h in range(8):
                            AT = A0T if i == 0 else A1N
                            nc.tensor.matmul(
                                pt[:, 48 * h:48 * (h + 1)],
                                AT[:, b * 8 + h, :],
                                YT[b][0:49, voc + 48 * h:voc + 48 * (h + 1)],
                                start=True, stop=True)
                        nc.vector.tensor_tensor(AGGf[32 * b:32 * b + 4, i, :], pt[:],
                                                VCB[64:68, b, i, :], OP.add)

                # agg norms -> AGGN (garbage partitions 4-31 harmless)
                SQ = st.tile([36, 768], F32, tag="sq")
                nc.gpsimd.tensor_tensor(SQ[:], AGGf[:].rearrange("p a c -> p (a c)"),
                                        AGGf[:].rearrange("p a c -> p (a c)"),
                                        OP.mult)
                SS = st.tile([36, 16], F32, tag="ss")
                nc.vector.reduce_sum(SS[:], SQ[:].rearrange("p (g c) -> p g c", c=48),
                                     axis=AX.X)
                nc.scalar.activation(SS[:], SS[:], AF.Sqrt)
                nc.vector.tensor_scalar_add(SS[:], SS[:], EPS)
                nc.vector.reciprocal(SS[:], SS[:])
                AGGN = SQ
                nc.gpsimd.tensor_tensor(
                    AGGN[:].rearrange("p (g c) -> p g c", c=48),
                    AGGf[:].rearrange("p a (g c) -> p (a g) c", c=48),
                    SS[:, :, None].to_broadcast((36, 16, 48)), OP.mult)

                # AGGN slices -> c-on-partitions AGGNT [48, (b,h), 8]
                AGGNT = st.tile([48, 16, 8], F32, tag="aggnt")
                for b in range(2):
                    for i in range(2):
                        for h in range(8):
                            pt = ps.tile([48, 4], F32, tag="S")
                            nc.tensor.transpose(
                                pt[:],
                                AGGN[32 * b:32 * b + 4, 384 * i + 48 * h:384 * i + 48 * (h + 1)],
                                ident[32 * b:32 * b + 4, 32 * b:32 * b + 4])
                            dst = AGGNT[:, b * 8 + h, 4 * i:4 * (i + 1)]
                            if si % 2 == 0:
                                nc.scalar.copy(dst, pt[:])
                            else:
                                nc.vector.tensor_copy(dst, pt[:])
                            si += 1

                # sim^T [49, (b,h,m)] = p^T @ aggnT ; rp / alpha / beta
                ptm = ps.tile([49, 128], F32, tag="M")
                for b in range(2):
                    for h in range(8):
                        s = b * 8 + h
                        nc.tensor.matmul(ptm[:, 8 * s:8 * (s + 1)],
                                         kslot(b, 2, h)[:, 0:49],
                                         AGGNT[:, s, :], start=True, stop=True)
                SIMT = st.tile([49, 16, 8], F32, tag="simt")
                nc.vector.tensor_tensor(SIMT[:],
                                        ptm[:].rearrange("p (a b) -> p a b", b=8),
                                        RP[:, :, None].to_broadcast((49, 16, 8)),
                                        OP.mult)
                nc.gpsimd.tensor_tensor(SIMT[:].rearrange("p a b -> p (a b)"),
                                        SIMT[:].rearrange("p a b -> p (a b)"),
                                        ALB[:], OP.mult)
                nc.gpsimd.tensor_tensor(SIMT[:].rearrange("p a b -> p (a b)"),
                                        SIMT[:].rearrange("p a b -> p (a b)"),
                                        BEB[:], OP.add)

                # assignment softmax over the 8 clusters, in place
                MM = st.tile([49, 16], F32, tag="mm")
                nc.vector.reduce_max(MM[:], SIMT[:], axis=AX.X, negate=True)
                nc.vector.tensor_tensor(SIMT[:], SIMT[:],
                                        MM[:, :, None].to_broadcast((49, 16, 8)),
                                        OP.add)
                nc.scalar.activation(SIMT[:], SIMT[:], AF.Exp)
                SMS = st.tile([49, 16], F32, tag="sms")
                nc.vector.reduce_sum(SMS[:], SIMT[:], axis=AX.X)
                nc.vector.reciprocal(SMS[:], SMS[:])
                ASGT = SIMT
                nc.gpsimd.tensor_tensor(ASGT[:], SIMT[:],
                                        SMS[:, :, None].to_broadcast((49, 16, 8)),
                                        OP.mult)

                # assignment -> m-on-partitions (b at base 32b)
                ASG = st.tile([36, 2, 8, 49], F32, tag="asg")
                for s in range(16):
                    b, h = divmod(s, 8)
                    for i in range(2):
                        pt = ps.tile([4, 49], F32, tag="S")
                        nc.tensor.transpose(pt[:], ASGT[:, s, 4 * i:4 * (i + 1)],
                                            ident[:49, :49])
                        dst = ASG[32 * b:32 * b + 4, i, h, :]
                        if si % 2 == 0:
                            nc.scalar.copy(dst, pt[:])
                        else:
                            nc.vector.tensor_copy(dst, pt[:])
                        si += 1

                # out_low z [48, h, (b*49)] = agg^T @ assignment  (bf16 out)
                Zf = st.tile([48, 8, 98], BF16, tag="zf")
                for b in range(2):
                    for h in range(8):
                        pt = ps.tile([48, 49], F32, tag="S")
                        nc.tensor.matmul(
                            pt[:],
                            AGGf[32 * b:32 * b + 4, 0, 48 * h:48 * (h + 1)],
                            ASG[32 * b:32 * b + 4, 0, h, :],
                            start=True, stop=False)
                        nc.tensor.matmul(
                            pt[:],
                            AGGf[32 * b:32 * b + 4, 1, 48 * h:48 * (h + 1)],
                            ASG[32 * b:32 * b + 4, 1, h, :],
                            start=False, stop=True)
                        dst = Zf[:, h, 49 * b:49 * (b + 1)]
                        if si % 2 == 0:
                            nc.scalar.copy(dst, pt[:])
                        else:
                            nc.vector.tensor_copy(dst, pt[:])
                        si += 1

                # proj2 transposed: z2T_b [49, 384] = Zf_b^T @ P2W (+bias)
                Z2T = []
                for b in range(2):
                    pt = ps.tile([49, 384], F32, tag="M")
                    for h in range(8):
                        nc.tensor.matmul(pt[:],
                                         Zf[:, h, 49 * b:49 * (b + 1)],
                                         P2W[:, h, :],
                                         start=(h == 0), stop=(h == 7))
                    z2t = st.tile([49, 384], BF16, tag=f"z2t{b}")
                    nc.vector.tensor_tensor(z2t[:], pt[:], P2B[:], OP.add)
                    Z2T.append(z2t)

                if stage == "noup":
                    ri = rep % 8
                    nc.sync.dma_start(y_d[0, 0:49, 384 * ri:384 * (ri + 1)],
                                      Z2T[0][:])
                    continue

                # upsample: [128, 3136] = z2T^T @ MUP (bf16), stage via SBUF
                for b in range(2):
                    for oi in range(3):
                        OUT = outp.tile([128, 3136], BF16, tag="out")
                        for nt in range(7):
                            pt = ps.tile([128, 448], F32, tag="U")
                            nc.tensor.matmul(
                                pt[:],
                                Z2T[b][:, 128 * oi:128 * (oi + 1)],
                                MUP[:, 448 * nt:448 * (nt + 1)],
                                start=True, stop=True)
                            dst = OUT[:, 448 * nt:448 * (nt + 1)]
                            if si % 2 == 0:
                                nc.scalar.copy(dst, pt[:])
                            else:
                                nc.vector.tensor_copy(dst, pt[:])
                            si += 1
                        nc.sync.dma_start(y_d[b, 128 * oi:128 * (oi + 1), :],
                                          OUT[:])

    nc.compile()
    return nc


def _prep_weights(proj_w, proj_b, sim_alpha, sim_beta, proj2_w, proj2_b):
    pwT = np.ascontiguousarray(proj_w.T) / 64.0                # [384, 1920]
    pwt = np.ascontiguousarray(pwT.reshape(3, 128, 1920).transpose(1, 0, 2))
    pbp = np.broadcast_to(proj_b[None, 0:384], (49, 384)).copy()
    # slot bias cols: 0-15 k0 (b0 then b1), 16-31 k1, 32-47 p
    pbns = np.zeros((48, 48), np.float32)
    for b in range(2):
        for h in range(8):
            pbns[:, b * 16 + h] = proj_b[384 + 48 * h:384 + 48 * (h + 1)]
            pbns[:, b * 16 + 8 + h] = proj_b[1152 + 48 * h:1152 + 48 * (h + 1)]
            pbns[:, 32 + b * 8 + h] = proj_b[48 * h:48 * (h + 1)]
    pbv = np.zeros((68, 2, 384), np.float32)
    pbv[64:68, 0, :] = 2.0 * proj_b[768:1152][None, :]
    pbv[64:68, 1, :] = 2.0 * proj_b[1536:1920][None, :]
    alph = np.tile(sim_alpha, 16)[None, :].astype(np.float32)
    beta = np.tile(sim_beta, 16)[None, :].astype(np.float32)
    p2wT = np.ascontiguousarray(proj2_w.T)                     # [384, 384]
    p2w = np.ascontiguousarray(p2wT.reshape(8, 48, 384).transpose(1, 0, 2))
    p2b = np.broadcast_to(proj2_b[None, :], (49, 384)).copy()
    mup = _upsample_matrix()
    return {"pwt": pwt.astype(ml_dtypes.bfloat16), "pbp": pbp, "pbns": pbns,
            "pbv": np.ascontiguousarray(pbv), "alph": alph, "beta": beta,
            "p2w": p2w.astype(ml_dtypes.bfloat16), "p2b": p2b,
            "mup": mup.astype(ml_dtypes.bfloat16)}


def kernel(x, proj_w, proj_b, sim_alpha, sim_beta, proj2_w, proj2_b):
    x = np.asarray(x, np.float32)
    proj_w = np.asarray(proj_w, np.float32)
    proj_b = np.asarray(proj_b, np.float32)
    sim_alpha = np.asarray(sim_alpha, np.float32)
    sim_beta = np.asarray(sim_beta, np.float32)
    proj2_w = np.asarray(proj2_w, np.float32)
    proj2_b = np.asarray(proj2_b, np.float32)

    if "nc" not in _CACHE:
        _CACHE["nc"] = build_nc()
    nc = _CACHE["nc"]

    w = _prep_weights(proj_w, proj_b, sim_alpha, sim_beta, proj2_w, proj2_b)
    B = x.shape[0]
    xr = x.reshape(8, B // 8, 384, 3136)
    in_maps = [dict(w, x=np.ascontiguousarray(xr[c])) for c in range(8)]

    res = run_bass_kernel_spmd(nc, in_maps, core_ids=list(range(8)))
    out = np.concatenate([np.asarray(r["y"]) for r in res.results], axis=0)
    return out.reshape(16, 384, 56, 56).astype(np.float32)


if __name__ == "__main__":
    rng = np.random.default_rng(0)
    inputs = {
        "x": rng.standard_normal((16, 384, 56, 56), dtype=np.float32),
        "proj_w": rng.standard_normal((1920, 384), dtype=np.float32) * 384 ** -0.5,
        "proj_b": rng.standard_normal(1920).astype(np.float32) * 0.1,
        "sim_alpha": np.ones(8, np.float32),
        "sim_beta": np.zeros(8, np.float32),
        "proj2_w": rng.standard_normal((384, 384), dtype=np.float32) * 384 ** -0.5,
        "proj2_b": rng.standard_normal(384).astype(np.float32) * 0.1,
    }
    out = kernel(**inputs)
    print("kernel ran, output", out.shape, out.dtype, float(np.abs(out).max()))
